# revision 46
# baseline (speedup 1.0000x reference)
"""CoAttention kernel for 8 Trainium2 NeuronCores.

Reference math (per batch item b, all fp32):
    aff  = tanh(q @ (v @ Waff.T + baff).T)            [NQ, NV]
    pv   = v @ Wv.T + bv                              [NV, HA]
    pq   = q @ Wq.T + bq                              [NQ, HA]
    h_v  = tanh(pv + aff.T @ pq)                      [NV, HA]
    h_q  = tanh(pq + aff @ pv)                        [NQ, HA]
    a_v  = softmax(h_v @ Whv.T + bhv, axis=0)         [NV, 1]
    a_q  = softmax(h_q @ Whq.T + bhq, axis=0)         [NQ, 1]
    v_hat = a_v.T @ v                                 [1, H]
    q_hat = a_q.T @ q                                 [1, H]

Strategy: pure data parallel — batch B=64 sharded 8 items/core, weights
replicated. bf16 matmuls with fp32 PSUM accumulation.

Per-core kernel structure (all layouts partition-major [128, ...]):
  - aff is computed via the associativity rewrite
        aff = tanh((q @ Waff) @ v.T + (q @ baff) 1^T)
    so the big [NV,H] intermediate v@Waff.T is never formed; the bias
    term is a per-partition ACT bias.
  - aff is needed with BOTH partition layouts (contraction over q for
    h_v, over n for h_q); the second layout comes from a DRAM roundtrip:
    aff is written out in an n-chunked layout and read back through the
    DMA xbar transpose as four [2048,128]->[128,2048] slabs.
  - pv/pq are computed transposed ([HA, *], bias = per-partition), cast
    to bf16; their natural layouts come from PE transposes (batched
    through PSUM with one DVE copy per 4 blocks).
  - The emission order software-pipelines items: item b's h_v/h_q and
    softmax tails are emitted after item b+1's front, so the PE stream
    never blocks on the affT roundtrip latency.
  - h_v/h_q are accumulated directly in PSUM: identity-matmul injects
    pvT/pqT, then the aff contraction accumulates on top; ACT applies
    tanh on the way out.
  - softmax never subtracts the max (logits bounded by sum|Whv| ~ 2.5;
    bhv/bhq shift all logits equally and cancel) and is folded into the
    final contraction: out = [sum_n e_n * v_n, sum_n e_n] via a ones
    column appended to v; the division happens on host.

The harness calls kernel(**inputs) with the full fp32 inputs and gets
back the full (v_hat, q_hat) fp32 tuple.
"""

import ml_dtypes
import numpy as np

import concourse.bass as bass
import concourse.mybir as mybir
import concourse.tile as tile
from concourse import bacc
from concourse.bass import ts
from concourse.bass_utils import run_bass_kernel_spmd
from concourse.masks import make_identity

B, NV, NQ, H, HA = 64, 2048, 512, 256, 128
NCORES = 8
BPC = B // NCORES  # batch items per core
P = 128
KH = H // P  # 2 k-tiles over H
TV = NV // P  # 16 partition tiles over NV
TQ = NQ // P  # 4 partition tiles over NQ
NCH = NV // 512  # 4 free-dim chunks of 512 over NV

BF = mybir.dt.bfloat16
F32 = mybir.dt.float32
Tanh = mybir.ActivationFunctionType.Tanh
Exp = mybir.ActivationFunctionType.Exp

_CACHE: dict = {}


def _build():
    nc = bacc.Bacc("TRN2", target_bir_lowering=False, debug=False)

    def din(name, shape, dt=BF):
        return nc.dram_tensor(name, shape, dt, kind="ExternalInput").ap()

    vT = din("vT", [BPC, H, NV])          # v[b].T
    v1 = din("v1", [BPC, NV, H + 1])      # v[b] with ones column
    qT = din("qT", [BPC, H, NQ])          # q[b].T
    q1 = din("q1", [BPC, NQ, H + 1])      # q[b] with ones column
    waff = din("waff", [H, H])            # Waff, natural layout
    wvT = din("wvT", [H, HA])             # Wv.T
    wqT = din("wqT", [H, HA])             # Wq.T
    whvT = din("whvT", [HA, 1])           # Whv.T
    whqT = din("whqT", [HA, 1])           # Whq.T
    baff = din("baff", [P, KH])           # baff[k*128+p] at [p, k]
    bv = din("bv", [HA, 1], F32)
    bq = din("bq", [HA, 1], F32)
    uv_out = nc.dram_tensor("uv_out", [BPC, H + 1], F32, kind="ExternalOutput").ap()
    uq_out = nc.dram_tensor("uq_out", [BPC, H + 1], F32, kind="ExternalOutput").ap()

    with tile.TileContext(nc) as tc:
        _body(tc, vT, v1, qT, q1, waff, wvT, wqT, whvT, whqT, baff, bv, bq,
              uv_out, uq_out)
    nc.compile()
    return nc


def _body(tc, vT, v1, qT, q1, waff, wvT, wqT, whvT, whqT, baff, bv, bq,
          uv_out, uq_out):
    nc = tc.nc
    with (
        tc.tile_pool(name="const", bufs=1) as const,
        tc.tile_pool(name="vin", bufs=2) as vin,
        tc.tile_pool(name="work", bufs=2) as work,
        tc.tile_pool(name="dram", bufs=2, space="DRAM") as dram,
        tc.tile_pool(name="psC", bufs=6, space="PSUM") as psC,
        tc.tile_pool(name="psS", bufs=2, space="PSUM") as psS,
    ):
        # ---- constants / weights (loaded once) ----
        waff_sb = const.tile([P, KH, H], BF, tag="waff")
        nc.gpsimd.dma_start(out=waff_sb, in_=waff.rearrange("(k p) h -> p k h", p=P))
        wvT_sb = const.tile([P, KH, HA], BF, tag="wvT")
        nc.gpsimd.dma_start(out=wvT_sb, in_=wvT.rearrange("(k p) a -> p k a", p=P))
        wqT_sb = const.tile([P, KH, HA], BF, tag="wqT")
        nc.gpsimd.dma_start(out=wqT_sb, in_=wqT.rearrange("(k p) a -> p k a", p=P))
        whvT_sb = const.tile([HA, 1], BF, tag="whvT")
        nc.gpsimd.dma_start(out=whvT_sb, in_=whvT)
        whqT_sb = const.tile([HA, 1], BF, tag="whqT")
        nc.gpsimd.dma_start(out=whqT_sb, in_=whqT)
        baff_sb = const.tile([P, KH], BF, tag="baff")
        nc.gpsimd.dma_start(out=baff_sb, in_=baff)
        bv_sb = const.tile([HA, 1], F32, tag="bv")
        nc.gpsimd.dma_start(out=bv_sb, in_=bv)
        bq_sb = const.tile([HA, 1], F32, tag="bq")
        nc.gpsimd.dma_start(out=bq_sb, in_=bq)
        ident = const.tile([P, P], BF, tag="ident")
        make_identity(nc, ident)

        def front(b):
            """Inputs, projections, aff, and the affT DRAM roundtrip."""
            st = {}
            # ---- per-item inputs ----
            vT_sb = vin.tile([P, KH, NV], BF, tag="vT")
            nc.scalar.dma_start(out=vT_sb, in_=vT[b].rearrange("(k p) n -> p k n", p=P))
            v1_sb = vin.tile([P, TV, H + 1], BF, tag="v1", bufs=3)
            nc.sync.dma_start(out=v1_sb, in_=v1[b].rearrange("(t p) c -> p t c", p=P))
            qT_sb = vin.tile([P, KH, NQ], BF, tag="qT")
            nc.scalar.dma_start(out=qT_sb, in_=qT[b].rearrange("(k p) n -> p k n", p=P))
            q1_sb = vin.tile([P, TQ, H + 1], BF, tag="q1", bufs=3)
            nc.sync.dma_start(out=q1_sb, in_=q1[b].rearrange("(t p) c -> p t c", p=P))

            # ---- qWT[h, q] = (q @ Waff).T = Waff.T @ qT ----
            qWT_sb = work.tile([P, KH, NQ], BF, tag="qWT")
            for m in range(KH):
                ps = psC.tile([P, NQ], F32, tag="chunk")
                for k in range(KH):
                    nc.tensor.matmul(ps, lhsT=waff_sb[:, k, ts(m, P)],
                                     rhs=qT_sb[:, k, :],
                                     start=(k == 0), stop=(k == KH - 1))
                nc.vector.tensor_copy(qWT_sb[:, m, :], ps)

            # ---- cq[q] = q @ baff (per-partition bias for aff tanh) ----
            cps = psS.tile([P, TQ], F32, tag="small")
            for m in range(TQ):
                for k in range(KH):
                    nc.tensor.matmul(cps[:, m : m + 1],
                                     lhsT=qT_sb[:, k, ts(m, P)],
                                     rhs=baff_sb[:, k : k + 1],
                                     start=(k == 0), stop=(k == KH - 1))
            cq_sb = work.tile([P, TQ], F32, tag="cq")
            nc.vector.tensor_copy(cq_sb, cps)

            # ---- pvT[a, n] = Wv @ v.T + bv ; pqT[a, q] = Wq @ q.T + bq ----
            pvT_sb = work.tile([P, NV], BF, tag="pvT")
            for c in range(NCH):
                ps = psC.tile([P, 512], F32, tag="chunk")
                for k in range(KH):
                    nc.tensor.matmul(ps, lhsT=wvT_sb[:, k, :],
                                     rhs=vT_sb[:, k, ts(c, 512)],
                                     start=(k == 0), stop=(k == KH - 1))
                nc.vector.tensor_scalar_add(pvT_sb[:, ts(c, 512)], ps, bv_sb)
            pqT_sb = work.tile([P, NQ], BF, tag="pqT")
            ps = psC.tile([P, NQ], F32, tag="chunk")
            for k in range(KH):
                nc.tensor.matmul(ps, lhsT=wqT_sb[:, k, :], rhs=qT_sb[:, k, :],
                                 start=(k == 0), stop=(k == KH - 1))
            nc.vector.tensor_scalar_add(pqT_sb, ps, bq_sb)

            # ---- natural layouts of pv/pq via PE transpose ----
            pvn_sb = work.tile([P, TV, HA], BF, tag="pvn")
            for g in range(TV // 4):
                tp = psC.tile([P, 4 * P], BF, tag="chunk")
                for j in range(4):
                    nc.tensor.transpose(tp[:, ts(j, P)],
                                        pvT_sb[:, ts(4 * g + j, P)], ident)
                nc.vector.tensor_copy(pvn_sb[:, 4 * g : 4 * g + 4, :], tp)
            pqn_sb = work.tile([P, TQ, HA], BF, tag="pqn")
            tp = psC.tile([P, 4 * P], BF, tag="chunk")
            for j in range(TQ):
                nc.tensor.transpose(tp[:, ts(j, P)], pqT_sb[:, ts(j, P)], ident)
            nc.vector.tensor_copy(pqn_sb, tp)

            # ---- aff[q, n] = tanh(qWT.T @ vT + cq), n-half-outer so the
            #      affT roundtrip for half 0 (writes + transposed reads)
            #      fires at the phase midpoint instead of the end ----
            aff_sb = work.tile([P, TQ, NV], BF, tag="aff")
            aff_dram = dram.tile([TV, NQ, P], BF, tag="aff_dram")
            affT_sb = work.tile([P, TV, NQ], BF, tag="affT")
            quart = TV // 4
            for nh in range(2):
                for m in range(TQ):
                    apss = [psC.tile([P, 512], F32, tag="chunk",
                                     name=f"aps{nh}{m}{c}") for c in range(2)]
                    for k in range(KH):
                        for c in range(2):
                            nc.tensor.matmul(apss[c],
                                             lhsT=qWT_sb[:, k, ts(m, P)],
                                             rhs=vT_sb[:, k, ts(2 * nh + c, 512)],
                                             start=(k == 0), stop=(k == KH - 1))
                    for c in range(2):
                        nc.scalar.activation(out=aff_sb[:, m, ts(2 * nh + c, 512)],
                                             in_=apss[c], func=Tanh,
                                             bias=cq_sb[:, m : m + 1])
                    nc.gpsimd.dma_start(
                        out=aff_dram[ts(nh, TV // 2), ts(m, P), :]
                        .rearrange("t p i -> p t i"),
                        in_=aff_sb[:, m, ts(nh, NV // 2)]
                        .rearrange("p (t i) -> p t i", i=P),
                    )
                for j in (2 * nh, 2 * nh + 1):
                    nc.sync.dma_start(
                        out=affT_sb[:, ts(j, quart), :],
                        in_=aff_dram[ts(j, quart)].rearrange("t q i -> (t q) i"),
                        transpose=True,
                    )
            st.update(vT_sb=vT_sb, v1_sb=v1_sb, q1_sb=q1_sb, aff_sb=aff_sb,
                      affT_sb=affT_sb, pvT_sb=pvT_sb, pqT_sb=pqT_sb,
                      pvn_sb=pvn_sb, pqn_sb=pqn_sb)
            return st

        def tail_hv(b, st):
            """h_v."""
            aff_sb = st["aff_sb"]
            pvT_sb = st["pvT_sb"]
            pqn_sb = st["pqn_sb"]

            # ---- hvT[a, n] = tanh(pvT + pq.T @ aff) ----
            hvT_sb = work.tile([P, NV], BF, tag="hvT", bufs=3)
            for c in range(NCH):
                hps = psC.tile([P, 512], F32, tag="chunk")
                nc.tensor.matmul(hps, lhsT=ident, rhs=pvT_sb[:, ts(c, 512)],
                                 start=True, stop=False)
                for k in range(TQ):
                    nc.tensor.matmul(hps, lhsT=pqn_sb[:, k, :],
                                     rhs=aff_sb[:, k, ts(c, 512)],
                                     start=False, stop=(k == TQ - 1))
                nc.scalar.activation(out=hvT_sb[:, ts(c, 512)], in_=hps, func=Tanh)

            st["hvT_sb"] = hvT_sb

        def tail_sv(b, st):
            """v-side softmax numerator/denominator, output DMA."""
            hvT_sb = st["hvT_sb"]
            v1_sb = st["v1_sb"]
            # ---- v logits + exp: ev = exp(hvT.T @ Whv.T) ----
            svps = psS.tile([P, TV], F32, tag="small")
            for t in range(TV):
                nc.tensor.matmul(svps[:, t : t + 1], lhsT=hvT_sb[:, ts(t, P)],
                                 rhs=whvT_sb, start=True, stop=True)
            ev_sb = work.tile([P, TV], BF, tag="ev")
            nc.scalar.activation(out=ev_sb, in_=svps, func=Exp)

            # ---- u_v = [sum_n e_n * v_n , sum_n e_n] via ones column ----
            uvps = psS.tile([1, H + 1], F32, tag="small")
            for t in range(TV):
                nc.tensor.matmul(uvps, lhsT=ev_sb[:, t : t + 1], rhs=v1_sb[:, t, :],
                                 start=(t == 0), stop=(t == TV - 1))
            uv_sb = work.tile([1, H + 1], F32, tag="uv")
            nc.vector.tensor_copy(uv_sb, uvps)
            nc.gpsimd.dma_start(out=uv_out[b : b + 1, :], in_=uv_sb)

        def tail_hq(b, st):
            """h_q (needs affT)."""
            affT_sb = st["affT_sb"]
            pqT_sb = st["pqT_sb"]
            pvn_sb = st["pvn_sb"]

            # ---- hqT[a, q] = tanh(pqT + pv.T @ affT) ----
            hqT_sb = work.tile([P, NQ], BF, tag="hqT", bufs=3)
            hps = psC.tile([P, NQ], F32, tag="chunk")
            nc.tensor.matmul(hps, lhsT=ident, rhs=pqT_sb, start=True, stop=False)
            for k in range(TV):
                nc.tensor.matmul(hps, lhsT=pvn_sb[:, k, :], rhs=affT_sb[:, k, :],
                                 start=False, stop=(k == TV - 1))
            nc.scalar.activation(out=hqT_sb, in_=hps, func=Tanh)

            st["hqT_sb"] = hqT_sb

        def tail_sq(b, st):
            """q-side softmax numerator/denominator, output DMA."""
            hqT_sb = st["hqT_sb"]
            q1_sb = st["q1_sb"]
            sqps = psS.tile([P, TQ], F32, tag="small")
            for t in range(TQ):
                nc.tensor.matmul(sqps[:, t : t + 1], lhsT=hqT_sb[:, ts(t, P)],
                                 rhs=whqT_sb, start=True, stop=True)
            eq_sb = work.tile([P, TQ], BF, tag="eq")
            nc.scalar.activation(out=eq_sb, in_=sqps, func=Exp)

            uqps = psS.tile([1, H + 1], F32, tag="small")
            for t in range(TQ):
                nc.tensor.matmul(uqps, lhsT=eq_sb[:, t : t + 1], rhs=q1_sb[:, t, :],
                                 start=(t == 0), stop=(t == TQ - 1))
            uq_sb = work.tile([1, H + 1], F32, tag="uq")
            nc.vector.tensor_copy(uq_sb, uqps)
            nc.gpsimd.dma_start(out=uq_out[b : b + 1, :], in_=uq_sb)

        # Software pipeline: emit front(b), then tail_v(b-1), then
        # tail_q(b-2) — tail_q consumes the affT DRAM roundtrip, which
        # thereby gets two full front-phases of latency slack.
        # Software pipeline: h_v/h_q of item b-1 run after front(b); the
        # serial softmax/output latency chains (sv->exp->u->DMA) are
        # deferred one more step so they never gate the PE stream.
        sts: dict[int, dict] = {}
        for b in range(BPC):
            sts[b] = front(b)
            if b >= 1:
                tail_hv(b - 1, sts[b - 1])
            if b >= 2:
                tail_sv(b - 2, sts[b - 2])
                tail_sq(b - 2, sts[b - 2])
            if b >= 1:
                tail_hq(b - 1, sts[b - 1])
        tail_hv(BPC - 1, sts[BPC - 1])
        tail_sv(BPC - 2, sts[BPC - 2])
        tail_sq(BPC - 2, sts[BPC - 2])
        tail_hq(BPC - 1, sts[BPC - 1])
        tail_sv(BPC - 1, sts[BPC - 1])
        tail_sq(BPC - 1, sts[BPC - 1])


def _prep_in_maps(v, q, Waff, baff, Wv, bv, Wq, bq, Whv, bhv, Whq, bhq):
    bf16 = ml_dtypes.bfloat16
    vb = v.astype(bf16)
    qb = q.astype(bf16)
    ones_v = np.ones((B, NV, 1), bf16)
    ones_q = np.ones((B, NQ, 1), bf16)
    vT = np.ascontiguousarray(vb.transpose(0, 2, 1))
    v1 = np.ascontiguousarray(np.concatenate([vb, ones_v], axis=2))
    qT = np.ascontiguousarray(qb.transpose(0, 2, 1))
    q1 = np.ascontiguousarray(np.concatenate([qb, ones_q], axis=2))
    shared = {
        "waff": np.ascontiguousarray(Waff.astype(bf16)),
        "wvT": np.ascontiguousarray(Wv.T.astype(bf16)),
        "wqT": np.ascontiguousarray(Wq.T.astype(bf16)),
        "whvT": np.ascontiguousarray(Whv.T.astype(bf16)),
        "whqT": np.ascontiguousarray(Whq.T.astype(bf16)),
        "baff": np.ascontiguousarray(baff.reshape(KH, P).T.astype(bf16)),
        "bv": np.ascontiguousarray(bv.reshape(HA, 1).astype(np.float32)),
        "bq": np.ascontiguousarray(bq.reshape(HA, 1).astype(np.float32)),
    }
    in_maps = []
    for c in range(NCORES):
        s = slice(c * BPC, (c + 1) * BPC)
        in_maps.append({
            "vT": np.ascontiguousarray(vT[s]),
            "v1": np.ascontiguousarray(v1[s]),
            "qT": np.ascontiguousarray(qT[s]),
            "q1": np.ascontiguousarray(q1[s]),
            **shared,
        })
    return in_maps


def _run(in_maps, trace=False, **kwargs):
    if "nc" not in _CACHE:
        _CACHE["nc"] = _build()
    return run_bass_kernel_spmd(
        _CACHE["nc"], in_maps, core_ids=list(range(NCORES)), trace=trace, **kwargs
    )


def _assemble(results):
    v_hat = np.zeros((B, 1, H), np.float32)
    q_hat = np.zeros((B, 1, H), np.float32)
    for c in range(NCORES):
        uv = results[c]["uv_out"]
        uq = results[c]["uq_out"]
        s = slice(c * BPC, (c + 1) * BPC)
        v_hat[s, 0, :] = uv[:, :H] / uv[:, H : H + 1]
        q_hat[s, 0, :] = uq[:, :H] / uq[:, H : H + 1]
    return v_hat, q_hat


def kernel(**inputs):
    inputs = {k: np.asarray(v) for k, v in inputs.items()}
    in_maps = _prep_in_maps(**inputs)
    res = _run(in_maps)
    return _assemble(res.results)


# revision 47
# speedup vs baseline: 1.3071x; 1.3071x over previous
"""CoAttention kernel for 8 Trainium2 NeuronCores.

Reference math (per batch item b, all fp32):
    aff  = tanh(q @ (v @ Waff.T + baff).T)            [NQ, NV]
    pv   = v @ Wv.T + bv                              [NV, HA]
    pq   = q @ Wq.T + bq                              [NQ, HA]
    h_v  = tanh(pv + aff.T @ pq)                      [NV, HA]
    h_q  = tanh(pq + aff @ pv)                        [NQ, HA]
    a_v  = softmax(h_v @ Whv.T + bhv, axis=0)         [NV, 1]
    a_q  = softmax(h_q @ Whq.T + bhq, axis=0)         [NQ, 1]
    v_hat = a_v.T @ v                                 [1, H]
    q_hat = a_q.T @ q                                 [1, H]

Strategy: pure data parallel — batch B=64 sharded 8 items/core, weights
replicated. bf16 matmuls with fp32 PSUM accumulation.

Per-core kernel structure (all layouts partition-major [128, ...]):
  - aff is computed via the associativity rewrite
        aff = tanh((q @ Waff) @ v.T + (q @ baff) 1^T)
    so the big [NV,H] intermediate v@Waff.T is never formed; the bias
    term is a per-partition ACT bias.
  - aff is needed with BOTH partition layouts (contraction over q for
    h_v, over n for h_q); the second layout comes from a DRAM roundtrip:
    aff is written out in an n-chunked layout and read back through the
    DMA xbar transpose as four [2048,128]->[128,2048] slabs.
  - pv/pq are computed transposed ([HA, *], bias = per-partition), cast
    to bf16; their natural layouts come from PE transposes (batched
    through PSUM with one DVE copy per 4 blocks).
  - The emission order software-pipelines items: item b's h_v/h_q and
    softmax tails are emitted after item b+1's front, so the PE stream
    never blocks on the affT roundtrip latency.
  - h_v/h_q are accumulated directly in PSUM: identity-matmul injects
    pvT/pqT, then the aff contraction accumulates on top; ACT applies
    tanh on the way out.
  - softmax never subtracts the max (logits bounded by sum|Whv| ~ 2.5;
    bhv/bhq shift all logits equally and cancel) and is folded into the
    final contraction: out = [sum_n e_n * v_n, sum_n e_n] via a ones
    column appended to v; the division happens on host.

The harness calls kernel(**inputs) with the full fp32 inputs and gets
back the full (v_hat, q_hat) fp32 tuple.
"""

import ml_dtypes
import numpy as np

import concourse.bass as bass
import concourse.mybir as mybir
import concourse.tile as tile
from concourse import bacc
from concourse.bass import ts
from concourse.bass_utils import run_bass_kernel_spmd
from concourse.masks import make_identity

B, NV, NQ, H, HA = 64, 2048, 512, 256, 128
NCORES = 8
BPC = B // NCORES  # batch items per core
P = 128
KH = H // P  # 2 k-tiles over H
TV = NV // P  # 16 partition tiles over NV
TQ = NQ // P  # 4 partition tiles over NQ
NCH = NV // 512  # 4 free-dim chunks of 512 over NV

BF = mybir.dt.bfloat16
F32 = mybir.dt.float32
Tanh = mybir.ActivationFunctionType.Tanh
Exp = mybir.ActivationFunctionType.Exp

_CACHE: dict = {}


def _build():
    nc = bacc.Bacc("TRN2", target_bir_lowering=False, debug=False)

    def din(name, shape, dt=BF):
        return nc.dram_tensor(name, shape, dt, kind="ExternalInput").ap()

    vT = din("vT", [BPC, H, NV])          # v[b].T
    v1 = din("v1", [BPC, NV, H + 1])      # v[b] with ones column
    qT = din("qT", [BPC, H, NQ])          # q[b].T
    q1 = din("q1", [BPC, NQ, H + 1])      # q[b] with ones column
    waff = din("waff", [H, H])            # Waff, natural layout
    wvT = din("wvT", [H, HA])             # Wv.T
    wqT = din("wqT", [H, HA])             # Wq.T
    whvT = din("whvT", [HA, 1])           # Whv.T
    whqT = din("whqT", [HA, 1])           # Whq.T
    baff = din("baff", [P, KH])           # baff[k*128+p] at [p, k]
    bv = din("bv", [HA, 1], F32)
    bq = din("bq", [HA, 1], F32)
    uv_out = nc.dram_tensor("uv_out", [BPC, H + 1], F32, kind="ExternalOutput").ap()
    uq_out = nc.dram_tensor("uq_out", [BPC, H + 1], F32, kind="ExternalOutput").ap()

    with tile.TileContext(nc) as tc:
        _body(tc, vT, v1, qT, q1, waff, wvT, wqT, whvT, whqT, baff, bv, bq,
              uv_out, uq_out)
    nc.compile()
    return nc


def _body(tc, vT, v1, qT, q1, waff, wvT, wqT, whvT, whqT, baff, bv, bq,
          uv_out, uq_out):
    nc = tc.nc
    with (
        tc.tile_pool(name="const", bufs=1) as const,
        tc.tile_pool(name="vin", bufs=2) as vin,
        tc.tile_pool(name="work", bufs=2) as work,
        tc.tile_pool(name="dram", bufs=2, space="DRAM") as dram,
        tc.tile_pool(name="psC", bufs=6, space="PSUM") as psC,
        tc.tile_pool(name="psS", bufs=2, space="PSUM") as psS,
    ):
        # ---- constants / weights (loaded once) ----
        waff_sb = const.tile([P, KH, H], BF, tag="waff")
        nc.gpsimd.dma_start(out=waff_sb, in_=waff.rearrange("(k p) h -> p k h", p=P))
        wvT_sb = const.tile([P, KH, HA], BF, tag="wvT")
        nc.gpsimd.dma_start(out=wvT_sb, in_=wvT.rearrange("(k p) a -> p k a", p=P))
        wqT_sb = const.tile([P, KH, HA], BF, tag="wqT")
        nc.gpsimd.dma_start(out=wqT_sb, in_=wqT.rearrange("(k p) a -> p k a", p=P))
        whvT_sb = const.tile([HA, 1], BF, tag="whvT")
        nc.gpsimd.dma_start(out=whvT_sb, in_=whvT)
        whqT_sb = const.tile([HA, 1], BF, tag="whqT")
        nc.gpsimd.dma_start(out=whqT_sb, in_=whqT)
        baff_sb = const.tile([P, KH], BF, tag="baff")
        nc.gpsimd.dma_start(out=baff_sb, in_=baff)
        bv_sb = const.tile([HA, 1], F32, tag="bv")
        nc.gpsimd.dma_start(out=bv_sb, in_=bv)
        bq_sb = const.tile([HA, 1], F32, tag="bq")
        nc.gpsimd.dma_start(out=bq_sb, in_=bq)
        ident = const.tile([P, P], BF, tag="ident")
        make_identity(nc, ident)

        def front(b):
            """Inputs, projections, aff, and the affT DRAM roundtrip."""
            st = {}
            # ---- per-item inputs ----
            vT_sb = vin.tile([P, KH, NV], BF, tag="vT")
            nc.scalar.dma_start(out=vT_sb, in_=vT[b].rearrange("(k p) n -> p k n", p=P))
            v1_sb = vin.tile([P, TV, H + 1], BF, tag="v1", bufs=3)
            nc.sync.dma_start(out=v1_sb, in_=v1[b].rearrange("(t p) c -> p t c", p=P))
            qT_sb = vin.tile([P, KH, NQ], BF, tag="qT")
            nc.scalar.dma_start(out=qT_sb, in_=qT[b].rearrange("(k p) n -> p k n", p=P))
            q1_sb = vin.tile([P, TQ, H + 1], BF, tag="q1", bufs=3)
            nc.sync.dma_start(out=q1_sb, in_=q1[b].rearrange("(t p) c -> p t c", p=P))

            # ---- qWT[h, q] = (q @ Waff).T = Waff.T @ qT ----
            qWT_sb = work.tile([P, KH, NQ], BF, tag="qWT")
            for m in range(KH):
                ps = psC.tile([P, NQ], F32, tag="chunk")
                for k in range(KH):
                    nc.tensor.matmul(ps, lhsT=waff_sb[:, k, ts(m, P)],
                                     rhs=qT_sb[:, k, :],
                                     start=(k == 0), stop=(k == KH - 1))
                nc.vector.tensor_copy(qWT_sb[:, m, :], ps)

            # ---- cq[q] = q @ baff (per-partition bias for aff tanh) ----
            cps = psS.tile([P, TQ], F32, tag="small")
            for m in range(TQ):
                for k in range(KH):
                    nc.tensor.matmul(cps[:, m : m + 1],
                                     lhsT=qT_sb[:, k, ts(m, P)],
                                     rhs=baff_sb[:, k : k + 1],
                                     start=(k == 0), stop=(k == KH - 1))
            cq_sb = work.tile([P, TQ], F32, tag="cq")
            nc.vector.tensor_copy(cq_sb, cps)

            # ---- pvT[a, n] = Wv @ v.T + bv ; pqT[a, q] = Wq @ q.T + bq ----
            pvT_sb = work.tile([P, NV], BF, tag="pvT")
            for c in range(NCH):
                ps = psC.tile([P, 512], F32, tag="chunk")
                for k in range(KH):
                    nc.tensor.matmul(ps, lhsT=wvT_sb[:, k, :],
                                     rhs=vT_sb[:, k, ts(c, 512)],
                                     start=(k == 0), stop=(k == KH - 1))
                nc.vector.tensor_scalar_add(pvT_sb[:, ts(c, 512)], ps, bv_sb)
            pqT_sb = work.tile([P, NQ], BF, tag="pqT")
            ps = psC.tile([P, NQ], F32, tag="chunk")
            for k in range(KH):
                nc.tensor.matmul(ps, lhsT=wqT_sb[:, k, :], rhs=qT_sb[:, k, :],
                                 start=(k == 0), stop=(k == KH - 1))
            nc.vector.tensor_scalar_add(pqT_sb, ps, bq_sb)

            # ---- natural layouts of pv/pq via PE transpose ----
            pvn_sb = work.tile([P, TV, HA], BF, tag="pvn")
            for g in range(TV // 4):
                tp = psC.tile([P, 4 * P], BF, tag="chunk")
                for j in range(4):
                    nc.tensor.transpose(tp[:, ts(j, P)],
                                        pvT_sb[:, ts(4 * g + j, P)], ident)
                nc.vector.tensor_copy(pvn_sb[:, 4 * g : 4 * g + 4, :], tp)
            pqn_sb = work.tile([P, TQ, HA], BF, tag="pqn")
            tp = psC.tile([P, 4 * P], BF, tag="chunk")
            for j in range(TQ):
                nc.tensor.transpose(tp[:, ts(j, P)], pqT_sb[:, ts(j, P)], ident)
            nc.vector.tensor_copy(pqn_sb, tp)

            # ---- aff[q, n] = tanh(qWT.T @ vT + cq) ----
            # k-outer: one weight load serves all 4 chunks back-to-back
            aff_sb = work.tile([P, TQ, NV], BF, tag="aff")
            for m in range(TQ):
                apss = [psC.tile([P, 512], F32, tag="chunk", name=f"aps{m}{c}")
                        for c in range(NCH)]
                for k in range(KH):
                    for c in range(NCH):
                        nc.tensor.matmul(apss[c],
                                         lhsT=qWT_sb[:, k, ts(m, P)],
                                         rhs=vT_sb[:, k, ts(c, 512)],
                                         start=(k == 0), stop=(k == KH - 1))
                for c in range(NCH):
                    nc.scalar.activation(out=aff_sb[:, m, ts(c, 512)],
                                         in_=apss[c], func=Tanh,
                                         bias=cq_sb[:, m : m + 1])

            # ---- affT[n, q] via DRAM roundtrip (n-chunked layout so the
            #      transposed load is one contiguous [512,128] slab per tile)
            aff_dram = dram.tile([TV, NQ, P], BF, tag="aff_dram")
            for m in range(TQ):
                nc.gpsimd.dma_start(
                    out=aff_dram[:, ts(m, P), :].rearrange("t p i -> p t i"),
                    in_=aff_sb[:, m, :].rearrange("p (t i) -> p t i", i=P),
                )
            affT_sb = work.tile([P, TV, NQ], BF, tag="affT")
            quart = TV // 4
            for j in range(4):
                nc.sync.dma_start(
                    out=affT_sb[:, ts(j, quart), :],
                    in_=aff_dram[ts(j, quart)].rearrange("t q i -> (t q) i"),
                    transpose=True,
                )
            st.update(vT_sb=vT_sb, v1_sb=v1_sb, q1_sb=q1_sb, aff_sb=aff_sb,
                      affT_sb=affT_sb, pvT_sb=pvT_sb, pqT_sb=pqT_sb,
                      pvn_sb=pvn_sb, pqn_sb=pqn_sb)
            return st

        def tail_hv(b, st):
            """h_v."""
            aff_sb = st["aff_sb"]
            pvT_sb = st["pvT_sb"]
            pqn_sb = st["pqn_sb"]

            # ---- hvT[a, n] = tanh(pvT + pq.T @ aff) ----
            hvT_sb = work.tile([P, NV], BF, tag="hvT", bufs=3)
            for c in range(NCH):
                hps = psC.tile([P, 512], F32, tag="chunk")
                nc.tensor.matmul(hps, lhsT=ident, rhs=pvT_sb[:, ts(c, 512)],
                                 start=True, stop=False)
                for k in range(TQ):
                    nc.tensor.matmul(hps, lhsT=pqn_sb[:, k, :],
                                     rhs=aff_sb[:, k, ts(c, 512)],
                                     start=False, stop=(k == TQ - 1))
                nc.scalar.activation(out=hvT_sb[:, ts(c, 512)], in_=hps, func=Tanh)

            st["hvT_sb"] = hvT_sb

        def tail_sv(b, st):
            """v-side softmax numerator/denominator, output DMA."""
            hvT_sb = st["hvT_sb"]
            v1_sb = st["v1_sb"]
            # ---- v logits + exp: ev = exp(hvT.T @ Whv.T) ----
            svps = psS.tile([P, TV], F32, tag="small")
            for t in range(TV):
                nc.tensor.matmul(svps[:, t : t + 1], lhsT=hvT_sb[:, ts(t, P)],
                                 rhs=whvT_sb, start=True, stop=True)
            ev_sb = work.tile([P, TV], BF, tag="ev")
            nc.scalar.activation(out=ev_sb, in_=svps, func=Exp)

            # ---- u_v = [sum_n e_n * v_n , sum_n e_n] via ones column ----
            uvps = psS.tile([1, H + 1], F32, tag="small")
            for t in range(TV):
                nc.tensor.matmul(uvps, lhsT=ev_sb[:, t : t + 1], rhs=v1_sb[:, t, :],
                                 start=(t == 0), stop=(t == TV - 1))
            uv_sb = work.tile([1, H + 1], F32, tag="uv")
            nc.vector.tensor_copy(uv_sb, uvps)
            nc.gpsimd.dma_start(out=uv_out[b : b + 1, :], in_=uv_sb)

        def tail_hq(b, st):
            """h_q (needs affT)."""
            affT_sb = st["affT_sb"]
            pqT_sb = st["pqT_sb"]
            pvn_sb = st["pvn_sb"]

            # ---- hqT[a, q] = tanh(pqT + pv.T @ affT) ----
            hqT_sb = work.tile([P, NQ], BF, tag="hqT", bufs=3)
            hps = psC.tile([P, NQ], F32, tag="chunk")
            nc.tensor.matmul(hps, lhsT=ident, rhs=pqT_sb, start=True, stop=False)
            for k in range(TV):
                nc.tensor.matmul(hps, lhsT=pvn_sb[:, k, :], rhs=affT_sb[:, k, :],
                                 start=False, stop=(k == TV - 1))
            nc.scalar.activation(out=hqT_sb, in_=hps, func=Tanh)

            st["hqT_sb"] = hqT_sb

        def tail_sq(b, st):
            """q-side softmax numerator/denominator, output DMA."""
            hqT_sb = st["hqT_sb"]
            q1_sb = st["q1_sb"]
            sqps = psS.tile([P, TQ], F32, tag="small")
            for t in range(TQ):
                nc.tensor.matmul(sqps[:, t : t + 1], lhsT=hqT_sb[:, ts(t, P)],
                                 rhs=whqT_sb, start=True, stop=True)
            eq_sb = work.tile([P, TQ], BF, tag="eq")
            nc.scalar.activation(out=eq_sb, in_=sqps, func=Exp)

            uqps = psS.tile([1, H + 1], F32, tag="small")
            for t in range(TQ):
                nc.tensor.matmul(uqps, lhsT=eq_sb[:, t : t + 1], rhs=q1_sb[:, t, :],
                                 start=(t == 0), stop=(t == TQ - 1))
            uq_sb = work.tile([1, H + 1], F32, tag="uq")
            nc.vector.tensor_copy(uq_sb, uqps)
            nc.gpsimd.dma_start(out=uq_out[b : b + 1, :], in_=uq_sb)

        # Software pipeline: emit front(b), then tail_v(b-1), then
        # tail_q(b-2) — tail_q consumes the affT DRAM roundtrip, which
        # thereby gets two full front-phases of latency slack.
        # Software pipeline: h_v/h_q of item b-1 run after front(b); the
        # serial softmax/output latency chains (sv->exp->u->DMA) are
        # deferred one more step so they never gate the PE stream.
        sts: dict[int, dict] = {}
        for b in range(BPC):
            sts[b] = front(b)
            if b >= 1:
                tail_hv(b - 1, sts[b - 1])
            if b >= 2:
                tail_sv(b - 2, sts[b - 2])
                tail_sq(b - 2, sts[b - 2])
            if b >= 1:
                tail_hq(b - 1, sts[b - 1])
        tail_hv(BPC - 1, sts[BPC - 1])
        tail_sv(BPC - 2, sts[BPC - 2])
        tail_sq(BPC - 2, sts[BPC - 2])
        tail_hq(BPC - 1, sts[BPC - 1])
        tail_sv(BPC - 1, sts[BPC - 1])
        tail_sq(BPC - 1, sts[BPC - 1])


def _prep_in_maps(v, q, Waff, baff, Wv, bv, Wq, bq, Whv, bhv, Whq, bhq):
    bf16 = ml_dtypes.bfloat16
    vb = v.astype(bf16)
    qb = q.astype(bf16)
    ones_v = np.ones((B, NV, 1), bf16)
    ones_q = np.ones((B, NQ, 1), bf16)
    vT = np.ascontiguousarray(vb.transpose(0, 2, 1))
    v1 = np.ascontiguousarray(np.concatenate([vb, ones_v], axis=2))
    qT = np.ascontiguousarray(qb.transpose(0, 2, 1))
    q1 = np.ascontiguousarray(np.concatenate([qb, ones_q], axis=2))
    shared = {
        "waff": np.ascontiguousarray(Waff.astype(bf16)),
        "wvT": np.ascontiguousarray(Wv.T.astype(bf16)),
        "wqT": np.ascontiguousarray(Wq.T.astype(bf16)),
        "whvT": np.ascontiguousarray(Whv.T.astype(bf16)),
        "whqT": np.ascontiguousarray(Whq.T.astype(bf16)),
        "baff": np.ascontiguousarray(baff.reshape(KH, P).T.astype(bf16)),
        "bv": np.ascontiguousarray(bv.reshape(HA, 1).astype(np.float32)),
        "bq": np.ascontiguousarray(bq.reshape(HA, 1).astype(np.float32)),
    }
    in_maps = []
    for c in range(NCORES):
        s = slice(c * BPC, (c + 1) * BPC)
        in_maps.append({
            "vT": np.ascontiguousarray(vT[s]),
            "v1": np.ascontiguousarray(v1[s]),
            "qT": np.ascontiguousarray(qT[s]),
            "q1": np.ascontiguousarray(q1[s]),
            **shared,
        })
    return in_maps


def _run(in_maps, trace=False, **kwargs):
    if "nc" not in _CACHE:
        _CACHE["nc"] = _build()
    return run_bass_kernel_spmd(
        _CACHE["nc"], in_maps, core_ids=list(range(NCORES)), trace=trace, **kwargs
    )


def _assemble(results):
    v_hat = np.zeros((B, 1, H), np.float32)
    q_hat = np.zeros((B, 1, H), np.float32)
    for c in range(NCORES):
        uv = results[c]["uv_out"]
        uq = results[c]["uq_out"]
        s = slice(c * BPC, (c + 1) * BPC)
        v_hat[s, 0, :] = uv[:, :H] / uv[:, H : H + 1]
        q_hat[s, 0, :] = uq[:, :H] / uq[:, H : H + 1]
    return v_hat, q_hat


def kernel(**inputs):
    inputs = {k: np.asarray(v) for k, v in inputs.items()}
    in_maps = _prep_in_maps(**inputs)
    res = _run(in_maps)
    return _assemble(res.results)
